# revision 11
# baseline (speedup 1.0000x reference)
"""Gaussian kernel message-passing (KernelSample) on 8 Trainium2 cores.

Math: out[i,:] = sum_j exp(-0.5*||x_i - y_j||^2) * [batch(i)==batch(j)] * w[j,:]

The batch mask makes K block-diagonal, so core b computes only batch b's
(2048 x 2048) block:   out_b = exp(-0.5 * d2(X_b, Y_b)) @ W_b

Per-core device pipeline (everything transposed so exp output feeds the
K@W contraction without a transpose):
  sq^T[ip,op] = aug_in(5,ip)^T @ aug_out(5,op)      (PE, float32r)
      aug_in  = [y0,y1,y2,|y|^2,1], aug_out = [-2x0,-2x1,-2x2,1,|x|^2]
  K^T = exp(-0.5*sq^T)                              (ACT, scale=-0.5)
  out^T[ch,op] += W_i(128,32)^T @ K^T_i(128,op)     (PE psum-accumulate)
"""

import os
from contextlib import ExitStack

import numpy as np

import concourse.bass as bass
import concourse.tile as tile
from concourse import mybir
from concourse.bass_utils import run_bass_kernel_spmd
from concourse.vector_clock import ScopedClock


class _SplitDrainTC(tile.TileContext):
    # The walrus CTRL struct supports few sync waits; spread the tail-drain
    # waits over extra sync-engine nops placed before the barrier.
    MAXW = 1

    def _drain_and_barrier(self, tick_clock, wait_clock):
        drain_inst = self.nc.sync.drain()
        wait_clock.add_sem_waits(
            drain_inst.ins, ScopedClock({None: tick_clock.global_clock}))
        si = drain_inst.ins.sync_info
        waits = list(si.on_wait) if si is not None else []
        if len(waits) > self.MAXW:
            si.on_wait = waits[:self.MAXW]
            rest = waits[self.MAXW:]
            while rest:
                chunk, rest = rest[:self.MAXW], rest[self.MAXW:]
                nop = self.nc.sync.nop(nofuse=True)
                nsi = nop.ins.sync_info
                if nsi is None:
                    nop.ins.sync_info = mybir.SyncInfo(
                        on_wait=chunk, on_update=[])
                else:
                    nsi.on_wait = chunk
        self.nc.all_engine_barrier()
        popped = self.nc._tile_sem_poison_stack.pop()
        assert popped is self._sem_poison
        self.nc.clear_and_free_semaphores(
            list(self.sems.allocated().values()))
        self.nc.all_engine_barrier()

N_CORES = 8
NB = 2048           # points per batch (both input and output side)
P = 128             # partitions
IP_TILES = NB // P  # 16 input-point tiles per core
CH = 32             # channels
OPH = 1024          # op-chunk (free dim) per sq psum tile: 2 PSUM banks
MM_DT = mybir.dt.float32r  # full-rate fp32 matmul mode
F32 = mybir.dt.float32

IN_PAD = 1.0e6      # padding position for unused input slots (K underflows to 0)
OUT_PAD = 2.0e6


def _build_nc():
    nc = bass.Bass("TRN2", target_bir_lowering=False, debug=False,
                   num_devices=N_CORES)
    ao = nc.dram_tensor("ao", [5, 2 * NB], MM_DT, kind="ExternalInput").ap()
    w = nc.dram_tensor("w", [P, IP_TILES * CH], MM_DT, kind="ExternalInput").ap()
    outT = nc.dram_tensor("outT", [CH, NB], F32, kind="ExternalOutput").ap()

    with _SplitDrainTC(nc) as tc:
        with ExitStack() as ctx:
            singles = ctx.enter_context(tc.tile_pool(name="singles", bufs=1))
            kt_pool = ctx.enter_context(tc.tile_pool(name="kt", bufs=IP_TILES * (NB // OPH)))
            sq_pool = ctx.enter_context(
                tc.tile_pool(name="sq", bufs=2, space="PSUM"))
            out_pool = ctx.enter_context(
                tc.tile_pool(name="out", bufs=1, space="PSUM"))
            ob_pool = ctx.enter_context(tc.tile_pool(name="ob", bufs=2))

            ao_sb = singles.tile([5, 2 * NB], MM_DT)
            nc.sync.dma_start(out=ao_sb, in_=ao)
            ain_sb = ao_sb[:, 0:NB]
            aout_sb = ao_sb[:, NB:2 * NB]
            w_sb = singles.tile([P, IP_TILES * CH], MM_DT)
            nc.sync.dma_start(out=w_sb, in_=w)

            psum_out = out_pool.tile([CH, NB], F32)

            # pre-touch: absorb the w DMA wait on a 1-elem matmul so the
            # first real K@W matmul carries only the ACT wait (LW struct
            # allows a single sync wait)
            nc.tensor.matmul(psum_out[0:1, 0:512], lhsT=w_sb[0:1, 0:1],
                             rhs=w_sb[0:1, 0:512], start=True, stop=True)

            for i in range(IP_TILES):
                lhs_ain = ain_sb[:, i * P:(i + 1) * P]
                w_i = w_sb[:, i * CH:(i + 1) * CH]
                for h in range(NB // OPH):
                    sq = sq_pool.tile([P, OPH], F32)
                    for q in range(OPH // 512):
                        nc.tensor.matmul(
                            sq[:, q * 512:(q + 1) * 512],
                            lhsT=lhs_ain,
                            rhs=aout_sb[:, h * OPH + q * 512:
                                        h * OPH + (q + 1) * 512],
                            start=True, stop=True)
                    kt = kt_pool.tile([P, OPH], MM_DT)
                    nc.scalar.activation(
                        out=kt, in_=sq,
                        func=mybir.ActivationFunctionType.Exp, scale=-0.5)
                    for q in range(OPH // 512):
                        nc.tensor.matmul(
                            psum_out[:, h * OPH + q * 512:
                                     h * OPH + (q + 1) * 512],
                            lhsT=w_i,
                            rhs=kt[:, q * 512:(q + 1) * 512],
                            start=(i == 0), stop=(i == IP_TILES - 1))

            for h in range(2):
                ob = ob_pool.tile([CH, NB // 2], F32)
                nc.vector.tensor_copy(
                    out=ob, in_=psum_out[:, h * (NB // 2):(h + 1) * (NB // 2)])
                nc.sync.dma_start(
                    out=outT[:, h * (NB // 2):(h + 1) * (NB // 2)], in_=ob)
    return nc


_NC_CACHE = None


def _get_nc():
    global _NC_CACHE
    if _NC_CACHE is None:
        _NC_CACHE = _build_nc()
    return _NC_CACHE


def _segments(lengths, total):
    off = np.concatenate([[0], np.cumsum(np.asarray(lengths, np.int64))])
    assert off[-1] == total, f"batch lengths sum {off[-1]} != {total}"
    return off


def make_in_maps(positions, weights, batch, output_positions, output_batch):
    positions = np.ascontiguousarray(positions, np.float32)
    weights = np.ascontiguousarray(weights, np.float32)
    output_positions = np.ascontiguousarray(output_positions, np.float32)
    in_off = _segments(batch, positions.shape[0])
    out_off = _segments(output_batch, output_positions.shape[0])
    n_batches = len(batch)
    assert n_batches == N_CORES and len(output_batch) == N_CORES

    in_maps = []
    for b in range(n_batches):
        y = positions[in_off[b]:in_off[b + 1]]
        wb = weights[in_off[b]:in_off[b + 1]]
        x = output_positions[out_off[b]:out_off[b + 1]]
        ni, no = y.shape[0], x.shape[0]
        assert ni <= NB and no <= NB

        ypad = np.full((NB, 3), IN_PAD, np.float32)
        ypad[:ni] = y
        wpad = np.zeros((NB, CH), np.float32)
        wpad[:ni] = wb
        xpad = np.full((NB, 3), OUT_PAD, np.float32)
        xpad[:no] = x

        ao = np.empty((5, 2 * NB), np.float32)
        ao[0:3, :NB] = ypad.T
        ao[3, :NB] = (ypad * ypad).sum(-1)
        ao[4, :NB] = 1.0
        ao[0:3, NB:] = -2.0 * xpad.T
        ao[3, NB:] = 1.0
        ao[4, NB:] = (xpad * xpad).sum(-1)
        wpk = np.ascontiguousarray(
            wpad.reshape(IP_TILES, P, CH).transpose(1, 0, 2).reshape(P, IP_TILES * CH))
        in_maps.append({"ao": ao, "w": wpk})
    return in_maps, out_off


def gather_output(results, out_off, n_out):
    out = np.empty((n_out, CH), np.float32)
    for b in range(N_CORES):
        no = out_off[b + 1] - out_off[b]
        out[out_off[b]:out_off[b + 1]] = results[b]["outT"].T[:no]
    return out


def run(inputs, trace=False, **kwargs):
    """Run on hardware; returns (full_output, BassKernelResults)."""
    nc = _get_nc()
    in_maps, out_off = make_in_maps(**inputs)
    res = run_bass_kernel_spmd(nc, in_maps, list(range(N_CORES)),
                               trace=trace, **kwargs)
    out = gather_output(res.results, out_off, inputs["output_positions"].shape[0])
    return out, res


def kernel(positions, weights, batch, output_positions, output_batch):
    out, _ = run(dict(positions=positions, weights=weights, batch=batch,
                      output_positions=output_positions,
                      output_batch=output_batch))
    return out


# revision 12
# speedup vs baseline: 1.1043x; 1.1043x over previous
"""Gaussian kernel message-passing (KernelSample) on 8 Trainium2 cores.

Math: out[i,:] = sum_j exp(-0.5*||x_i - y_j||^2) * [batch(i)==batch(j)] * w[j,:]

The batch mask makes K block-diagonal, so core b computes only batch b's
(2048 x 2048) block:   out_b = exp(-0.5 * d2(X_b, Y_b)) @ W_b

Per-core device pipeline (everything transposed so exp output feeds the
K@W contraction without a transpose):
  sq^T[ip,op] = aug_in^T @ aug_out                   (PE, bf16 hi/lo split)
      aug_in  = [y,|y|^2,1] as [hi;hi;lo] (15 rows)
      aug_out = [-2x,1,|x|^2] as [hi;lo;hi] (15 rows)
      => sq = a_hi.b_hi + a_hi.b_lo + a_lo.b_hi  (~fp32 accuracy, 1 cyc/row)
  K^T = exp(-0.5*sq^T)  -> fp16                      (ACT, scale=-0.5)
  out^T[ch,op] += W_i(128,32)^T @ K^T_i(128,op)      (PE fp16, psum-accum)
"""

from contextlib import ExitStack

import ml_dtypes
import numpy as np

import concourse.bass as bass
import concourse.tile as tile
from concourse import mybir
from concourse.bass_utils import run_bass_kernel_spmd
from concourse.vector_clock import ScopedClock

N_CORES = 8
NB = 2048           # points per batch (both input and output side)
P = 128             # partitions
IP_TILES = NB // P  # 16 input-point tiles per core
CH = 32             # channels
OPH = 1024          # op-chunk (free dim) per sq psum tile: 2 PSUM banks
F32 = mybir.dt.float32
BF16 = mybir.dt.bfloat16
FP16 = mybir.dt.float16

IN_PAD = 1.0e6      # padding position for unused input slots (K underflows to 0)
OUT_PAD = 2.0e6


class _SplitDrainTC(tile.TileContext):
    # The walrus CTRL struct supports few sync waits; spread the tail-drain
    # waits over extra sync-engine nops placed before the barrier.
    MAXW = 1

    def _drain_and_barrier(self, tick_clock, wait_clock):
        drain_inst = self.nc.sync.drain()
        wait_clock.add_sem_waits(
            drain_inst.ins, ScopedClock({None: tick_clock.global_clock}))
        si = drain_inst.ins.sync_info
        waits = list(si.on_wait) if si is not None else []
        if len(waits) > self.MAXW:
            si.on_wait = waits[:self.MAXW]
            rest = waits[self.MAXW:]
            while rest:
                chunk, rest = rest[:self.MAXW], rest[self.MAXW:]
                nop = self.nc.sync.nop(nofuse=True)
                nsi = nop.ins.sync_info
                if nsi is None:
                    nop.ins.sync_info = mybir.SyncInfo(
                        on_wait=chunk, on_update=[])
                else:
                    nsi.on_wait = chunk
        self.nc.all_engine_barrier()
        popped = self.nc._tile_sem_poison_stack.pop()
        assert popped is self._sem_poison
        self.nc.clear_and_free_semaphores(
            list(self.sems.allocated().values()))
        self.nc.all_engine_barrier()


def _build_nc():
    nc = bass.Bass("TRN2", target_bir_lowering=False, debug=False,
                   num_devices=N_CORES)
    ao = nc.dram_tensor("ao", [15, 2 * NB], BF16, kind="ExternalInput").ap()
    w = nc.dram_tensor("w", [P, IP_TILES * CH], FP16, kind="ExternalInput").ap()
    outT = nc.dram_tensor("outT", [CH, NB], F32, kind="ExternalOutput").ap()

    with _SplitDrainTC(nc) as tc:
        with ExitStack() as ctx:
            singles = ctx.enter_context(tc.tile_pool(name="singles", bufs=1))
            kt_pool = ctx.enter_context(
                tc.tile_pool(name="kt", bufs=IP_TILES * (NB // OPH)))
            sq_pool = ctx.enter_context(
                tc.tile_pool(name="sq", bufs=2, space="PSUM"))
            out_pool = ctx.enter_context(
                tc.tile_pool(name="out", bufs=2, space="PSUM"))
            ob_pool = ctx.enter_context(tc.tile_pool(name="ob", bufs=2))

            ao_sb = singles.tile([15, 2 * NB], BF16)
            nc.sync.dma_start(out=ao_sb, in_=ao)
            ain_sb = ao_sb[:, 0:NB]
            aout_sb = ao_sb[:, NB:2 * NB]
            w_sb = singles.tile([P, IP_TILES * CH], FP16)
            nc.sync.dma_start(out=w_sb, in_=w)

            first = True
            for h in range(NB // OPH):
                psum_out = out_pool.tile([CH, OPH], F32)
                if first:
                    # pre-touch: absorb the w DMA wait on a cheap matmul so
                    # the first real K@W matmul carries only the ACT wait
                    # (the matmul LW struct allows a single sync wait)
                    nc.tensor.matmul(psum_out[0:1, 0:512],
                                     lhsT=w_sb[0:1, 0:1],
                                     rhs=w_sb[0:1, 0:512],
                                     start=True, stop=True)
                    first = False
                for i in range(IP_TILES):
                    lhs_ain = ain_sb[:, i * P:(i + 1) * P]
                    w_i = w_sb[:, i * CH:(i + 1) * CH]
                    sq = sq_pool.tile([P, OPH], F32)
                    for q in range(OPH // 512):
                        nc.tensor.matmul(
                            sq[:, q * 512:(q + 1) * 512],
                            lhsT=lhs_ain,
                            rhs=aout_sb[:, h * OPH + q * 512:
                                        h * OPH + (q + 1) * 512],
                            start=True, stop=True)
                    kt = kt_pool.tile([P, OPH], FP16)
                    nc.scalar.activation(
                        out=kt, in_=sq,
                        func=mybir.ActivationFunctionType.Exp, scale=-0.5)
                    for q in range(OPH // 512):
                        nc.tensor.matmul(
                            psum_out[:, q * 512:(q + 1) * 512],
                            lhsT=w_i,
                            rhs=kt[:, q * 512:(q + 1) * 512],
                            start=(i == 0), stop=(i == IP_TILES - 1))
                ob = ob_pool.tile([CH, OPH], F32)
                nc.vector.tensor_copy(out=ob, in_=psum_out)
                nc.sync.dma_start(
                    out=outT[:, h * OPH:(h + 1) * OPH], in_=ob)
    return nc


_NC_CACHE = None


def _get_nc():
    global _NC_CACHE
    if _NC_CACHE is None:
        _NC_CACHE = _build_nc()
    return _NC_CACHE


def _segments(lengths, total):
    off = np.concatenate([[0], np.cumsum(np.asarray(lengths, np.int64))])
    assert off[-1] == total, f"batch lengths sum {off[-1]} != {total}"
    return off


def _hilo(a):
    """Split fp32 array into (hi, lo) bf16 pair with hi+lo ~= a."""
    hi = a.astype(ml_dtypes.bfloat16)
    lo = (a - hi.astype(np.float32)).astype(ml_dtypes.bfloat16)
    return hi, lo


def make_in_maps(positions, weights, batch, output_positions, output_batch):
    positions = np.ascontiguousarray(positions, np.float32)
    weights = np.ascontiguousarray(weights, np.float32)
    output_positions = np.ascontiguousarray(output_positions, np.float32)
    in_off = _segments(batch, positions.shape[0])
    out_off = _segments(output_batch, output_positions.shape[0])
    n_batches = len(batch)
    assert n_batches == N_CORES and len(output_batch) == N_CORES

    in_maps = []
    for b in range(n_batches):
        y = positions[in_off[b]:in_off[b + 1]]
        wb = weights[in_off[b]:in_off[b + 1]]
        x = output_positions[out_off[b]:out_off[b + 1]]
        ni, no = y.shape[0], x.shape[0]
        assert ni <= NB and no <= NB

        ypad = np.full((NB, 3), IN_PAD, np.float32)
        ypad[:ni] = y
        wpad = np.zeros((NB, CH), np.float32)
        wpad[:ni] = wb
        xpad = np.full((NB, 3), OUT_PAD, np.float32)
        xpad[:no] = x

        a = np.empty((5, NB), np.float32)       # aug_in (fp32)
        a[0:3] = ypad.T
        a[3] = (ypad * ypad).sum(-1)
        a[4] = 1.0
        bb = np.empty((5, NB), np.float32)      # aug_out (fp32)
        bb[0:3] = -2.0 * xpad.T
        bb[3] = 1.0
        bb[4] = (xpad * xpad).sum(-1)
        a_hi, a_lo = _hilo(a)
        b_hi, b_lo = _hilo(bb)

        # dot([a_hi;a_hi;a_lo], [b_hi;b_lo;b_hi]) =
        #   a_hi.b_hi + a_hi.b_lo + a_lo.b_hi ~= a.b
        ao = np.empty((15, 2 * NB), ml_dtypes.bfloat16)
        ao[0:5, :NB], ao[5:10, :NB], ao[10:15, :NB] = a_hi, a_hi, a_lo
        ao[0:5, NB:], ao[5:10, NB:], ao[10:15, NB:] = b_hi, b_lo, b_hi

        wpk = np.ascontiguousarray(
            wpad.reshape(IP_TILES, P, CH).transpose(1, 0, 2)
            .reshape(P, IP_TILES * CH)).astype(np.float16)
        in_maps.append({"ao": ao, "w": wpk})
    return in_maps, out_off


def gather_output(results, out_off, n_out):
    out = np.empty((n_out, CH), np.float32)
    for b in range(N_CORES):
        no = out_off[b + 1] - out_off[b]
        out[out_off[b]:out_off[b + 1]] = results[b]["outT"].T[:no]
    return out


def run(inputs, trace=False, **kwargs):
    """Run on hardware; returns (full_output, BassKernelResults)."""
    nc = _get_nc()
    in_maps, out_off = make_in_maps(**inputs)
    res = run_bass_kernel_spmd(nc, in_maps, list(range(N_CORES)),
                               trace=trace, **kwargs)
    out = gather_output(res.results, out_off, inputs["output_positions"].shape[0])
    return out, res


def kernel(positions, weights, batch, output_positions, output_batch):
    out, _ = run(dict(positions=positions, weights=weights, batch=batch,
                      output_positions=output_positions,
                      output_batch=output_batch))
    return out


# revision 13
# speedup vs baseline: 1.1089x; 1.0041x over previous
"""Gaussian kernel message-passing (KernelSample) on 8 Trainium2 cores.

Math: out[i,:] = sum_j exp(-0.5*||x_i - y_j||^2) * [batch(i)==batch(j)] * w[j,:]

The batch mask makes K block-diagonal, so core b computes only batch b's
(2048 x 2048) block:   out_b = exp(-0.5 * d2(X_b, Y_b)) @ W_b

Per-core device pipeline (everything transposed so exp output feeds the
K@W contraction without a transpose):
  sq^T[ip,op] = aug_in^T @ aug_out                   (PE, bf16 hi/lo split)
      aug_in  = [y,|y|^2,1] as [hi;hi;lo] (15 rows)
      aug_out = [-2x,1,|x|^2] as [hi;lo;hi] (15 rows)
      => sq = a_hi.b_hi + a_hi.b_lo + a_lo.b_hi  (~fp32 accuracy, 1 cyc/row)
  K^T = exp(-0.5*sq^T)  -> fp16                      (ACT, scale=-0.5)
  out^T[ch,op] += W_i(128,32)^T @ K^T_i(128,op)      (PE fp16, psum-accum)
"""

from contextlib import ExitStack

import ml_dtypes
import numpy as np

import concourse.bass as bass
import concourse.tile as tile
from concourse import mybir
from concourse.bass_utils import run_bass_kernel_spmd
from concourse.vector_clock import ScopedClock

N_CORES = 8
NB = 2048           # points per batch (both input and output side)
P = 128             # partitions
IP_TILES = NB // P  # 16 input-point tiles per core
CH = 32             # channels
OPH = 1024          # op-chunk (free dim) per sq psum tile: 2 PSUM banks
F32 = mybir.dt.float32
BF16 = mybir.dt.bfloat16
FP16 = mybir.dt.float16

IN_PAD = 1.0e6      # padding position for unused input slots (K underflows to 0)
OUT_PAD = 2.0e6


class _SplitDrainTC(tile.TileContext):
    # The walrus CTRL struct supports few sync waits; spread the tail-drain
    # waits over extra sync-engine nops placed before the barrier.
    MAXW = 1

    def _drain_and_barrier(self, tick_clock, wait_clock):
        drain_inst = self.nc.sync.drain()
        wait_clock.add_sem_waits(
            drain_inst.ins, ScopedClock({None: tick_clock.global_clock}))
        si = drain_inst.ins.sync_info
        waits = list(si.on_wait) if si is not None else []
        if len(waits) > self.MAXW:
            si.on_wait = waits[:self.MAXW]
            rest = waits[self.MAXW:]
            while rest:
                chunk, rest = rest[:self.MAXW], rest[self.MAXW:]
                nop = self.nc.sync.nop(nofuse=True)
                nsi = nop.ins.sync_info
                if nsi is None:
                    nop.ins.sync_info = mybir.SyncInfo(
                        on_wait=chunk, on_update=[])
                else:
                    nsi.on_wait = chunk
        self.nc.all_engine_barrier()
        popped = self.nc._tile_sem_poison_stack.pop()
        assert popped is self._sem_poison
        self.nc.clear_and_free_semaphores(
            list(self.sems.allocated().values()))
        self.nc.all_engine_barrier()


def _build_nc():
    nc = bass.Bass("TRN2", target_bir_lowering=False, debug=False,
                   num_devices=N_CORES)
    ao = nc.dram_tensor("ao", [15, 2 * NB], BF16, kind="ExternalInput").ap()
    w = nc.dram_tensor("w", [P, IP_TILES * CH], FP16, kind="ExternalInput").ap()
    outT = nc.dram_tensor("outT", [CH, NB], F32, kind="ExternalOutput").ap()

    with _SplitDrainTC(nc) as tc:
        with ExitStack() as ctx:
            singles = ctx.enter_context(tc.tile_pool(name="singles", bufs=1))
            kt_pool = ctx.enter_context(
                tc.tile_pool(name="kt", bufs=IP_TILES * (NB // OPH)))
            sq_pool = ctx.enter_context(
                tc.tile_pool(name="sq", bufs=2, space="PSUM"))
            out_pool = ctx.enter_context(
                tc.tile_pool(name="out", bufs=2, space="PSUM"))
            ob_pool = ctx.enter_context(tc.tile_pool(name="ob", bufs=2))

            ao_sb = singles.tile([15, 2 * NB], BF16)
            nc.sync.dma_start(out=ao_sb, in_=ao)
            ain_sb = ao_sb[:, 0:NB]
            aout_sb = ao_sb[:, NB:2 * NB]
            w_sb = singles.tile([P, IP_TILES * CH], FP16)
            nc.sync.dma_start(out=w_sb, in_=w)

            first = True
            for h in range(NB // OPH):
                psum_out = out_pool.tile([CH, OPH], F32)
                if first:
                    # pre-touch: absorb the w DMA wait on a cheap matmul so
                    # the first real K@W matmul carries only the ACT wait
                    # (the matmul LW struct allows a single sync wait)
                    nc.tensor.matmul(psum_out[0:1, 0:512],
                                     lhsT=w_sb[0:1, 0:1],
                                     rhs=w_sb[0:1, 0:512],
                                     start=True, stop=True)
                    first = False
                kts = [None] * IP_TILES
                # software pipeline: K@W for tile i-1 issues between the
                # distance matmuls of tile i and i+1, so PE never stalls
                # waiting for the exp of the tile it just produced.
                for idx in range(IP_TILES + 1):
                    if idx < IP_TILES:
                        i = idx
                        lhs_ain = ain_sb[:, i * P:(i + 1) * P]
                        sq = sq_pool.tile([P, OPH], F32)
                        for q in range(OPH // 512):
                            nc.tensor.matmul(
                                sq[:, q * 512:(q + 1) * 512],
                                lhsT=lhs_ain,
                                rhs=aout_sb[:, h * OPH + q * 512:
                                            h * OPH + (q + 1) * 512],
                                start=True, stop=True)
                        kt = kt_pool.tile([P, OPH], FP16)
                        nc.scalar.activation(
                            out=kt, in_=sq,
                            func=mybir.ActivationFunctionType.Exp, scale=-0.5)
                        kts[i] = kt
                    if idx > 0:
                        i = idx - 1
                        w_i = w_sb[:, i * CH:(i + 1) * CH]
                        for q in range(OPH // 512):
                            nc.tensor.matmul(
                                psum_out[:, q * 512:(q + 1) * 512],
                                lhsT=w_i,
                                rhs=kts[i][:, q * 512:(q + 1) * 512],
                                start=(i == 0), stop=(i == IP_TILES - 1))
                ob = ob_pool.tile([CH, OPH], F32)
                nc.vector.tensor_copy(out=ob, in_=psum_out)
                nc.sync.dma_start(
                    out=outT[:, h * OPH:(h + 1) * OPH], in_=ob)
    return nc


_NC_CACHE = None


def _get_nc():
    global _NC_CACHE
    if _NC_CACHE is None:
        _NC_CACHE = _build_nc()
    return _NC_CACHE


def _segments(lengths, total):
    off = np.concatenate([[0], np.cumsum(np.asarray(lengths, np.int64))])
    assert off[-1] == total, f"batch lengths sum {off[-1]} != {total}"
    return off


def _hilo(a):
    """Split fp32 array into (hi, lo) bf16 pair with hi+lo ~= a."""
    hi = a.astype(ml_dtypes.bfloat16)
    lo = (a - hi.astype(np.float32)).astype(ml_dtypes.bfloat16)
    return hi, lo


def make_in_maps(positions, weights, batch, output_positions, output_batch):
    positions = np.ascontiguousarray(positions, np.float32)
    weights = np.ascontiguousarray(weights, np.float32)
    output_positions = np.ascontiguousarray(output_positions, np.float32)
    in_off = _segments(batch, positions.shape[0])
    out_off = _segments(output_batch, output_positions.shape[0])
    n_batches = len(batch)
    assert n_batches == N_CORES and len(output_batch) == N_CORES

    in_maps = []
    for b in range(n_batches):
        y = positions[in_off[b]:in_off[b + 1]]
        wb = weights[in_off[b]:in_off[b + 1]]
        x = output_positions[out_off[b]:out_off[b + 1]]
        ni, no = y.shape[0], x.shape[0]
        assert ni <= NB and no <= NB

        ypad = np.full((NB, 3), IN_PAD, np.float32)
        ypad[:ni] = y
        wpad = np.zeros((NB, CH), np.float32)
        wpad[:ni] = wb
        xpad = np.full((NB, 3), OUT_PAD, np.float32)
        xpad[:no] = x

        a = np.empty((5, NB), np.float32)       # aug_in (fp32)
        a[0:3] = ypad.T
        a[3] = (ypad * ypad).sum(-1)
        a[4] = 1.0
        bb = np.empty((5, NB), np.float32)      # aug_out (fp32)
        bb[0:3] = -2.0 * xpad.T
        bb[3] = 1.0
        bb[4] = (xpad * xpad).sum(-1)
        a_hi, a_lo = _hilo(a)
        b_hi, b_lo = _hilo(bb)

        # dot([a_hi;a_hi;a_lo], [b_hi;b_lo;b_hi]) =
        #   a_hi.b_hi + a_hi.b_lo + a_lo.b_hi ~= a.b
        ao = np.empty((15, 2 * NB), ml_dtypes.bfloat16)
        ao[0:5, :NB], ao[5:10, :NB], ao[10:15, :NB] = a_hi, a_hi, a_lo
        ao[0:5, NB:], ao[5:10, NB:], ao[10:15, NB:] = b_hi, b_lo, b_hi

        wpk = np.ascontiguousarray(
            wpad.reshape(IP_TILES, P, CH).transpose(1, 0, 2)
            .reshape(P, IP_TILES * CH)).astype(np.float16)
        in_maps.append({"ao": ao, "w": wpk})
    return in_maps, out_off


def gather_output(results, out_off, n_out):
    out = np.empty((n_out, CH), np.float32)
    for b in range(N_CORES):
        no = out_off[b + 1] - out_off[b]
        out[out_off[b]:out_off[b + 1]] = results[b]["outT"].T[:no]
    return out


def run(inputs, trace=False, **kwargs):
    """Run on hardware; returns (full_output, BassKernelResults)."""
    nc = _get_nc()
    in_maps, out_off = make_in_maps(**inputs)
    res = run_bass_kernel_spmd(nc, in_maps, list(range(N_CORES)),
                               trace=trace, **kwargs)
    out = gather_output(res.results, out_off, inputs["output_positions"].shape[0])
    return out, res


def kernel(positions, weights, batch, output_positions, output_batch):
    out, _ = run(dict(positions=positions, weights=weights, batch=batch,
                      output_positions=output_positions,
                      output_batch=output_batch))
    return out
